# revision 12
# baseline (speedup 1.0000x reference)
"""Trainium2 Bass kernel for nn_Attention_65455301591248.

Multi-head attention: B=32, C=768, H=12 heads, S=512, D=64.
  q/k/v = W{q,k,v} @ x + b   (1x1 conv == channel GEMM), head-minor channel
  scores[k,h,q] = (q.k)/sqrt(D) + mask[k,q];  softmax over k
  attn = w @ v; concat head-major; out = Wo @ attn + bo
Sharding: pure data parallel over batch - 4 batches per core x 8 cores.

v3: flat cross-batch software pipeline (one stream of 24 (batch, pair)
steps instead of per-batch attention calls).
  - Lookahead-2 between scores and attn: attn for pair s-2 is emitted at
    step s, so by the time the PE reaches it the exp->mask-mul chain
    (ACT ~1us + DVE) has long finished -- removes the ~0.7-1us PE stall
    per pair the v2 schedule had.
  - A filler thunk is drained between scores kc1 and kc2 so the ps_s
    2-buffer recycle (scores kc2 waits for exp kc0 to read PSUM) is
    covered by independent projection matmuls.
  - Batch boundaries pipeline seamlessly (scores of batch b+1 pair 0/1
    are emitted before attn of batch b pairs 4/5), removing the ~1.6us
    boundary bubbles and the HAM half-clock penalty that followed them.
  - em (mask exp) is split into two thunks pushed mid-batch so the big
    ACT instructions don't sit in front of score exps at batch start.
  - Last batch: PE-broadcast norms per pair (DMA/gpsimd-free) and the
    final o_proj accumulation interleaved between the last attn pairs
    keep the PE streaming through the tail instead of idling (which
    also kept the tail at half PE clock in v2).
Same host-side prep as v2: weights pre-transposed head-major, Wq/bq
pre-scaled, bv folded into bo, mask exp scaled 2^-6, per-head ones
column accumulates the softmax denominator in PSUM row 64.
"""

from collections import deque

import numpy as np

try:
    import concourse.bass as bass  # noqa: F401
except ImportError:  # pragma: no cover
    import sys

    sys.path.insert(0, "/opt/trn_rl_repo")

import concourse.bass as bass
import concourse.tile as tile
from concourse import bacc, mybir
from concourse.bass_utils import run_bass_kernel_spmd

B, C, H, S, D = 32, 768, 12, 512, 64
NCORES = 8
NB = B // NCORES  # batches per core
F16 = mybir.dt.float16
F32 = mybir.dt.float32
NC_CHUNKS = C // 128  # 6
NK_CHUNKS = S // 128  # 4
NPAIR = H // 2  # 6
VROW = H * (D + 1)  # 780: per-head 64 v columns + 1 ones column
EM_BIAS = -6.0 * float(np.log(2.0))  # exp(mask)*2^-6

_COMPILED = None


def _build():
    """Build + compile the per-core Bass program (runs on each of 8 cores)."""
    nc = bacc.Bacc("TRN2", target_bir_lowering=False, debug=False)

    x_d = nc.dram_tensor("x", [NB, C, S], F16, kind="ExternalInput")
    m_d = nc.dram_tensor("mask", [NB, S, S], F16, kind="ExternalInput")
    wq_d = nc.dram_tensor("wqt", [C, C], F16, kind="ExternalInput")
    wk_d = nc.dram_tensor("wkt", [C, C], F16, kind="ExternalInput")
    wv_d = nc.dram_tensor("wvt", [C, C], F16, kind="ExternalInput")
    wo_d = nc.dram_tensor("wot", [C, C], F16, kind="ExternalInput")
    # packed per-partition biases: cols 0-5 bq/8, 6-11 bk, 12-17 bo',
    # col 18 = EM_BIAS constant
    bcol_d = nc.dram_tensor("bcols", [128, 19], F32, kind="ExternalInput")
    y_d = nc.dram_tensor("y", [NB, C, S], F16, kind="ExternalOutput")

    with tile.TileContext(nc) as tc:
        with (
            tc.tile_pool(name="wpool", bufs=1) as wpool,
            tc.tile_pool(name="const", bufs=1) as const,
            tc.tile_pool(name="xp", bufs=2) as xp,
            tc.tile_pool(name="qk", bufs=2) as qk,
            tc.tile_pool(name="vp", bufs=2) as vp,
            tc.tile_pool(name="mp", bufs=2) as mp,
            tc.tile_pool(name="wexp", bufs=3) as wexp,
            tc.tile_pool(name="stgp", bufs=2) as stgp,
            tc.tile_pool(name="cat", bufs=2) as cat,
            tc.tile_pool(name="op", bufs=1) as op,
            tc.tile_pool(name="rp", bufs=2) as rp,
            tc.tile_pool(name="ps_proj", bufs=2, space="PSUM") as ps_proj,
            tc.tile_pool(name="ps_s", bufs=2, space="PSUM") as ps_s,
            tc.tile_pool(name="ps_a", bufs=2, space="PSUM") as ps_a,
        ):
            # ---- persistent weights / constants -------------------------
            def load_w(w_d, name, chunked=False):
                t = wpool.tile([128, NC_CHUNKS * C], F16, tag=name, name=name)
                if chunked:
                    for j in range(NC_CHUNKS):
                        nc.sync.dma_start(
                            out=t[:, j * C : (j + 1) * C],
                            in_=w_d.ap()[j * 128 : (j + 1) * 128, :],
                        )
                else:
                    nc.sync.dma_start(
                        out=t.rearrange("p (j c) -> p j c", c=C),
                        in_=w_d.ap().rearrange("(j p) c -> p j c", p=128),
                    )
                return t

            bcol = const.tile([128, 19], F32, tag="bcol")
            nc.sync.dma_start(out=bcol[:], in_=bcol_d.ap()[:, :])

            def wview(t, ki):
                return t[:, ki * C : (ki + 1) * C]

            # ---- per-batch input loads ---------------------------------
            def load_x(b, chunked=True):
                t = xp.tile([128, NC_CHUNKS * S], F16, tag="x", name=f"x{b}")
                if chunked:
                    for j in range(NC_CHUNKS):
                        nc.sync.dma_start(
                            out=t[:, j * S : (j + 1) * S],
                            in_=x_d.ap()[b, j * 128 : (j + 1) * 128, :],
                        )
                else:
                    nc.sync.dma_start(
                        out=t.rearrange("p (j s) -> p j s", s=S),
                        in_=x_d.ap()[b].rearrange("(j p) s -> p j s", p=128),
                    )
                return t

            def load_mask(b):
                t = mp.tile([128, NK_CHUNKS * S], F16, tag="mraw", name=f"m{b}")
                nc.sync.dma_start(
                    out=t.rearrange("p (kc q) -> p kc q", q=S),
                    in_=m_d.ap()[b].rearrange("(kc p) q -> p kc q", p=128),
                )
                return t

            # ---- work queue --------------------------------------------
            work_q = deque()

            def drain1():
                if work_q:
                    work_q.popleft()()

            def drain_until(cond):
                while not cond():
                    assert work_q, "work exhausted before operand ready"
                    work_q.popleft()()

            def drain_all():
                while work_q:
                    work_q.popleft()()

            # em2 = exp(mask)*2^-6, each kc chunk duplicated so the es
            # multiply gets one contiguous [128,1024] operand per kc.
            # Split into two thunks (r=0/1) so the two big ACT
            # instructions can be spaced apart in the ACT queue.
            def em_thunks(mt, em_out, em_n):
                def one(r):
                    if em_out[0] is None:
                        em_out[0] = mp.tile(
                            [128, 2 * NK_CHUNKS * S], F16, tag="em", name="em"
                        )
                    ev = em_out[0].rearrange("p (kc r q) -> p kc r q", r=2, q=S)
                    mv = mt.rearrange("p (kc q) -> p kc q", q=S)
                    nc.scalar.activation(
                        out=ev[:, :, r, :],
                        in_=mv[:],
                        func=mybir.ActivationFunctionType.Exp,
                        bias=bcol[:, 18:19],
                    )
                    em_n[0] += 1

                return [lambda: one(0), lambda: one(1)]

            # ---- projection groups -------------------------------------
            def qk_group(w_t, xt, bias_col, name, co, outs):
                ps = ps_proj.tile([128, S], F32, tag="proj", name="ps_p")
                for ki in range(NC_CHUNKS):
                    nc.tensor.matmul(
                        ps[:],
                        wview(w_t, ki)[:, co * 128 : (co + 1) * 128],
                        xt[:, ki * S : (ki + 1) * S],
                        start=(ki == 0),
                        stop=(ki == NC_CHUNKS - 1),
                    )
                dt = qk.tile([128, S], F16, tag=f"{name}{co}", name=f"{name}{co}")
                if co % 2 == 0:
                    nc.vector.tensor_scalar_add(
                        dt[:], ps[:], bcol[:, bias_col + co : bias_col + co + 1]
                    )
                else:
                    nc.scalar.activation(
                        out=dt[:],
                        in_=ps[:],
                        func=mybir.ActivationFunctionType.Identity,
                        bias=bcol[:, bias_col + co : bias_col + co + 1],
                    )
                outs[co] = dt

            def v_group(wv_load, xt, sc, half, v_out):
                # v^T projection chunk: out [s, c'] with per-head ones col
                if half == 0:
                    vt = vp.tile([128, VROW], F16, tag=f"v{sc}", name=f"v{sc}")
                    vv = vt.rearrange("p (h w) -> p h w", w=D + 1)
                    nc.vector.memset(vv[:, :, D : D + 1], 1.0)
                    v_out[sc] = vt
                else:
                    vt = v_out[sc]
                    vv = vt.rearrange("p (h w) -> p h w", w=D + 1)
                v_out[(sc, half)] = True
                hw = C // 2  # 384 = 6 heads
                ps = ps_proj.tile([128, hw], F32, tag="proj", name="ps_v")
                for ki in range(NC_CHUNKS):
                    nc.tensor.matmul(
                        ps[:],
                        xt[:, ki * S + sc * 128 : ki * S + (sc + 1) * 128],
                        wview(wv_load[0], ki)[:, half * hw : (half + 1) * hw],
                        start=(ki == 0),
                        stop=(ki == NC_CHUNKS - 1),
                    )
                if half == 0:
                    nc.scalar.activation(
                        out=vv[:, 0:6, 0:D],
                        in_=ps.rearrange("p (h w) -> p h w", w=D),
                        func=mybir.ActivationFunctionType.Copy,
                    )
                else:
                    nc.vector.tensor_copy(
                        vv[:, 6:12, 0:D],
                        ps.rearrange("p (h w) -> p h w", w=D),
                    )

            def oproj_thunks(wo_load, b, cat_sb):
                def one(co):
                    ps = ps_proj.tile([128, S], F32, tag="proj", name="ps_o")
                    for ki in range(NC_CHUNKS):
                        nc.tensor.matmul(
                            ps[:],
                            wview(wo_load[0], ki)[:, co * 128 : (co + 1) * 128],
                            cat_sb[ki][:],
                            start=(ki == 0),
                            stop=(ki == NC_CHUNKS - 1),
                        )
                    ot = op.tile([128, S], F16, tag=f"o{co}", name="ot")
                    if co % 2 == 0:
                        nc.vector.tensor_scalar_add(
                            ot[:], ps[:], bcol[:, 12 + co : 13 + co]
                        )
                    else:
                        nc.scalar.activation(
                            out=ot[:],
                            in_=ps[:],
                            func=mybir.ActivationFunctionType.Identity,
                            bias=bcol[:, 12 + co : 13 + co],
                        )
                    nc.sync.dma_start(
                        out=y_d.ap()[b, co * 128 : (co + 1) * 128, :], in_=ot[:]
                    )

                return [lambda co=co: one(co) for co in range(NC_CHUNKS)]

            # ---- pipeline emitters -------------------------------------
            def make_ctx():
                return {
                    "q": {},
                    "k": {},
                    "v": {},
                    "em": [None],
                    "em_n": [0],
                    "cat": None,
                    "stg": None,
                }

            def ensure_catstg(ctx):
                if ctx["cat"] is None:
                    ctx["cat"] = [
                        cat.tile([128, S], F16, tag=f"c{j}", name=f"cat{j}")
                        for j in range(NC_CHUNKS)
                    ]
                    ctx["stg"] = [
                        stgp.tile([D + 1, 6 * S], F16, tag="stg0", name="stg0"),
                        stgp.tile([D + 1, 6 * S], F16, tag="stg1", name="stg1"),
                    ]

            def v_ready(ctx, hp):
                need = [(kc, 0) for kc in range(NK_CHUNKS)]
                if hp >= 3:
                    need += [(kc, 1) for kc in range(NK_CHUNKS)]
                return all(k in ctx["v"] for k in need)

            def emit_scores_pair(ctx, hp):
                em = ctx["em"][0]
                q_sb, k_sb = ctx["q"], ctx["k"]
                es_tiles = []
                for kc in range(NK_CHUNKS):
                    if kc == 2:
                        # filler absorbs the ps_s 2-buffer recycle wait
                        drain1()
                    ps = ps_s.tile([128, 2 * S], F32, tag="spair", name="ps_sc")
                    for j in range(2):
                        po = j * D
                        nc.tensor.matmul(
                            ps[:, j * S : (j + 1) * S],
                            k_sb[hp][po : po + D, kc * 128 : (kc + 1) * 128],
                            q_sb[hp][po : po + D, :],
                            start=True,
                            stop=True,
                            tile_position=(po, 0),
                        )
                    es = wexp.tile([128, 2 * S], F16, tag=f"es{kc}", name="es")
                    nc.scalar.activation(
                        out=es[:],
                        in_=ps[:],
                        func=mybir.ActivationFunctionType.Exp,
                    )
                    nc.vector.tensor_mul(
                        es[:], es[:], em[:, kc * 2 * S : (kc + 1) * 2 * S]
                    )
                    es_tiles.append(es)
                return es_tiles

            def emit_attn_pair(ctx, hp, es_tiles):
                ensure_catstg(ctx)
                stg, v_sb = ctx["stg"], ctx["v"]
                psas = []
                for j in range(2):
                    h = 2 * hp + j
                    psa = ps_a.tile([D + 1, S], F32, tag="attn", name="psa")
                    for kc in range(NK_CHUNKS - 1):
                        nc.tensor.matmul(
                            psa[:],
                            v_sb[kc][:, h * (D + 1) : (h + 1) * (D + 1)],
                            es_tiles[kc][:, j * S : (j + 1) * S],
                            start=(kc == 0),
                            stop=False,
                        )
                    psas.append(psa)
                drain1()
                kc = NK_CHUNKS - 1
                for j in range(2):
                    h = 2 * hp + j
                    psa = psas[j]
                    nc.tensor.matmul(
                        psa[:],
                        v_sb[kc][:, h * (D + 1) : (h + 1) * (D + 1)],
                        es_tiles[kc][:, j * S : (j + 1) * S],
                        start=False,
                        stop=True,
                    )
                    dst = stg[h // 6][:, (h % 6) * S : (h % 6 + 1) * S]
                    if j == 0:
                        nc.vector.tensor_copy(dst, psa[:])
                    else:
                        nc.scalar.activation(
                            out=dst,
                            in_=psa[:],
                            func=mybir.ActivationFunctionType.Copy,
                        )

            def emit_norm_half(ctx, half):
                sh = ctx["stg"][half]
                cat_sb = ctx["cat"]
                r12 = rp.tile([6, S], F16, tag="r12", name="r12")
                nc.sync.dma_start(out=r12[:], in_=sh[D : D + 1, :])
                r12f = rp.tile([6, S], F32, tag="r12f", name="r12f")
                nc.vector.tensor_copy(r12f[:], r12[:])
                rrf = rp.tile([6, S], F32, tag="rrf", name="rrf")
                nc.vector.reciprocal_approx_fast(out=rrf[:], in_=r12f[:])
                rr = rp.tile([6, S], F16, tag="rr", name="rr")
                nc.scalar.activation(
                    out=rr[:],
                    in_=rrf[:],
                    func=mybir.ActivationFunctionType.Copy,
                )
                rbsrc = rp.tile([1, 6 * S], F16, tag="rbs", name="rbs")
                nc.sync.dma_start(out=rbsrc[:], in_=rr[:])
                for jj in range(6):
                    h = 6 * half + jj
                    hp, po = h // 2, (h % 2) * D
                    rb = rp.tile([D, S], F16, tag=f"rb{jj % 2}", name="rb")
                    nc.gpsimd.partition_broadcast(
                        rb[:], rbsrc[0:1, jj * S : (jj + 1) * S]
                    )
                    nc.vector.tensor_mul(
                        cat_sb[hp][po : po + D, :],
                        sh[0:D, jj * S : (jj + 1) * S],
                        rb[:],
                    )

            def emit_norm_pair(ctx, hp):
                # DMA-free norm (last batch): PE broadcasts the denominator
                # row (K=1 matmul from partition 64), reciprocal in place
                # on PSUM, multiply.
                cat_sb = ctx["cat"]
                for j in range(2):
                    h = 2 * hp + j
                    sh = ctx["stg"][h // 6]
                    col = (h % 6) * S
                    psrb = ps_a.tile([D, S], F32, tag="attn", name="psrb")
                    nc.tensor.matmul(
                        psrb[:],
                        ones65[D : D + 1, :],
                        sh[D : D + 1, col : col + S],
                        start=True,
                        stop=True,
                    )
                    nc.vector.reciprocal_approx_fast(out=psrb[:], in_=psrb[:])
                    nc.vector.tensor_mul(
                        cat_sb[hp][j * D : (j + 1) * D, :],
                        sh[0:D, col : col + S],
                        psrb[:],
                    )

            # ---- prologue: warmup + batch-0 loads ----------------------
            # Dummy matmuls on a zeroed tile bridge the DMA-startup window
            # so the PE clock (HAM) is already warm when real work lands.
            dummy = const.tile([128, S], F16, tag="dummy")
            nc.vector.memset(dummy[:], 0.0)
            # ones column at partition 64 for the PE-broadcast tail norm
            ones65 = const.tile([D + 1, D], F16, tag="ones65")
            nc.vector.memset(ones65[:], 1.0)
            NDUMMY = 16
            for i in range(NDUMMY):
                psd = ps_proj.tile([128, S], F32, tag="proj", name="ps_d")
                nc.tensor.matmul(
                    psd[:], dummy[:, 0:128], dummy[:], start=True, stop=True
                )
                if i == NDUMMY - 1:
                    nc.vector.tensor_copy(dummy[:, 0:1], psd[:, 0:1])

            # DMA order: wq/x interleaved per chunk (matmul ki only waits
            # for chunk ki), then wk, mask0, wv, wo.
            wq_t = wpool.tile([128, NC_CHUNKS * C], F16, tag="wq", name="wq")
            xt0 = xp.tile([128, NC_CHUNKS * S], F16, tag="x", name="x0")
            for j in range(NC_CHUNKS):
                nc.sync.dma_start(
                    out=wq_t[:, j * C : (j + 1) * C],
                    in_=wq_d.ap()[j * 128 : (j + 1) * 128, :],
                )
                nc.sync.dma_start(
                    out=xt0[:, j * S : (j + 1) * S],
                    in_=x_d.ap()[0, j * 128 : (j + 1) * 128, :],
                )
            wk_t = load_w(wk_d, "wk", chunked=True)
            mt0 = load_mask(0)
            wv_load, wo_load = [None], [None]

            def loadwv():
                wv_load[0] = load_w(wv_d, "wv")

            def loadwo():
                wo_load[0] = load_w(wo_d, "wo")

            ctxs = {0: make_ctx()}
            ctxs[0]["xt"] = xt0
            # emit first q/k groups so scores pair 0 can start early
            qk_group(wq_t, xt0, 0, "q", 0, ctxs[0]["q"])
            qk_group(wk_t, xt0, 6, "k", 0, ctxs[0]["k"])

            def qg(ctx, co):
                return lambda: qk_group(wq_t, ctx["xt"], 0, "q", co, ctx["q"])

            def kg(ctx, co):
                return lambda: qk_group(wk_t, ctx["xt"], 6, "k", co, ctx["k"])

            def vg(ctx, sc, half):
                return lambda: v_group(wv_load, ctx["xt"], sc, half, ctx["v"])

            em0 = em_thunks(mt0, ctxs[0]["em"], ctxs[0]["em_n"])
            c0 = ctxs[0]
            work_q.extend(
                [
                    em0[0],
                    em0[1],
                    qg(c0, 1), kg(c0, 1), qg(c0, 2), kg(c0, 2),
                    loadwv,
                    vg(c0, 0, 0), vg(c0, 1, 0), vg(c0, 2, 0), vg(c0, 3, 0),
                    qg(c0, 3), kg(c0, 3), qg(c0, 4), kg(c0, 4),
                    vg(c0, 0, 1), vg(c0, 1, 1), vg(c0, 2, 1), vg(c0, 3, 1),
                    qg(c0, 5), kg(c0, 5),
                    loadwo,
                ]
            )

            # ---- flat pipeline over 24 (batch, pair) steps -------------
            pend = []
            reserve = []  # thunks held back to fill the last batch
            em_next_th = [None]
            oproj_pending = {}

            def oproj_wrapped(pb, cat_sb):
                def wrap(t):
                    def f():
                        t()
                        oproj_pending[pb] -= 1

                    return f

                return [wrap(t) for t in oproj_thunks(wo_load, pb, cat_sb)]
            for b in range(NB):
                ctx = ctxs[b]
                for hp in range(NPAIR):
                    if hp == 0 and b + 1 < NB:
                        nctx = make_ctx()
                        nctx["xt"] = load_x(b + 1)
                        nctx["mt"] = load_mask(b + 1)
                        ctxs[b + 1] = nctx
                    if hp == 0 and b == NB - 1:
                        work_q.extend(reserve)
                        reserve = []
                    if hp == 2 and b + 1 < NB:
                        work_q.append(em_next_th[0][0])
                    if hp == 3 and b + 1 < NB:
                        work_q.append(em_next_th[0][1])

                    drain_until(
                        lambda: hp in ctx["q"]
                        and hp in ctx["k"]
                        and ctx["em_n"][0] == 2
                    )
                    es = emit_scores_pair(ctx, hp)
                    pend.append((b, hp, es))
                    if len(pend) > 2:
                        pb, php, pes = pend.pop(0)
                        pctx = ctxs[pb]
                        if php == 0:
                            # cat/stg buffers of batch pb-2 must be fully
                            # consumed before ensure_catstg reallocates them
                            drain_until(
                                lambda: oproj_pending.get(pb - 2, 0) == 0
                            )
                        drain_until(lambda: v_ready(pctx, php))
                        emit_attn_pair(pctx, php, pes)
                        if pb == NB - 1:
                            drain1()
                            emit_norm_pair(pctx, php)
                        else:
                            if php == 2:
                                emit_norm_half(pctx, 0)
                            elif php == 5:
                                emit_norm_half(pctx, 1)
                                oproj_pending[pb] = NC_CHUNKS
                                work_q.extend(oproj_wrapped(pb, pctx["cat"]))
                    if hp == 1 and b + 1 < NB:
                        # push next batch's qkv work -- AFTER this step's
                        # pop so attn(b-1,5)'s reads of the old v/q/k tile
                        # instances are emitted before any reallocation
                        nctx = ctxs[b + 1]
                        if b + 1 == NB - 1:
                            # hold back work whose consumers come late --
                            # v half1 (attn pairs 3+) and q5/k5 -- so the
                            # last batch still has filler thunks
                            keep, hold = [], []
                            for co in range(NC_CHUNKS):
                                pair = [qg(nctx, co), kg(nctx, co)]
                                (hold if co == 5 else keep).extend(pair)
                            for sc in range(NK_CHUNKS):
                                keep.append(vg(nctx, sc, 0))
                            for sc in range(NK_CHUNKS):
                                hold.append(vg(nctx, sc, 1))
                            work_q.extend(keep)
                            reserve = hold
                        else:
                            th = []
                            for co in range(NC_CHUNKS):
                                th.append(qg(nctx, co))
                                th.append(kg(nctx, co))
                            for sc in range(NK_CHUNKS):
                                for half in range(2):
                                    th.append(vg(nctx, sc, half))
                            work_q.extend(th)
                        em_next_th[0] = em_thunks(
                            nctx["mt"], nctx["em"], nctx["em_n"]
                        )

            # ---- epilogue: last two attn pairs + final o_proj ----------
            drain_all()
            ctx3 = ctxs[NB - 1]
            cat3 = ctx3["cat"]
            wo_t = wo_load[0]
            # PSUM accumulators for the 6 output-channel chunks: co 0/1 on
            # ps_proj (free immediately), co 2-5 on ps_s views (free once
            # the last scores pair's exps have read them)
            pso = []
            for _ in range(2):
                pt = ps_proj.tile([128, S], F32, tag="proj", name="ps_fo2")
                pso.append(pt[:])
            for _ in range(2):
                pt = ps_s.tile([128, 2 * S], F32, tag="spair", name="ps_fo")
                pso.append(pt[:, 0:S])
                pso.append(pt[:, S : 2 * S])
            started = [False] * NC_CHUNKS

            def opartial(kis):
                for co in range(NC_CHUNKS):
                    for ki in kis:
                        nc.tensor.matmul(
                            pso[co],
                            wview(wo_t, ki)[:, co * 128 : (co + 1) * 128],
                            cat3[ki][:],
                            start=not started[co],
                            stop=False,
                        )
                        started[co] = True

            opartial([0, 1, 2])
            pb, php, pes = pend.pop(0)  # (NB-1, 4)
            emit_attn_pair(ctx3, php, pes)
            opartial([3])
            emit_norm_pair(ctx3, 4)
            pb, php, pes = pend.pop(0)  # (NB-1, 5)
            emit_attn_pair(ctx3, php, pes)
            opartial([4])
            emit_norm_pair(ctx3, 5)
            for co in range(NC_CHUNKS):
                nc.tensor.matmul(
                    pso[co],
                    wview(wo_t, 5)[:, co * 128 : (co + 1) * 128],
                    cat3[5][:],
                    start=False,
                    stop=True,
                )
                ot = op.tile([128, S], F16, tag=f"o{co}", name="ot")
                if co % 2 == 0:
                    nc.vector.tensor_scalar_add(
                        ot[:], pso[co], bcol[:, 12 + co : 13 + co]
                    )
                    nc.sync.dma_start(
                        out=y_d.ap()[NB - 1, co * 128 : (co + 1) * 128, :],
                        in_=ot[:],
                    )
                else:
                    nc.scalar.activation(
                        out=ot[:],
                        in_=pso[co],
                        func=mybir.ActivationFunctionType.Identity,
                        bias=bcol[:, 12 + co : 13 + co],
                    )
                    # issue from the ACT queue: SyncE is draining its own
                    # semaphores at the tail
                    nc.scalar.dma_start(
                        out=y_d.ap()[NB - 1, co * 128 : (co + 1) * 128, :],
                        in_=ot[:],
                    )

    nc.compile()
    return nc


def _get_compiled():
    global _COMPILED
    if _COMPILED is None:
        _COMPILED = _build()
    return _COMPILED


def _headmajor(wT):
    """Permute the output-channel axis of a transposed weight from the
    reference's head-minor order (c = d*H + h) to head-major (c' = h*D + d)."""
    return np.ascontiguousarray(
        wT.reshape(C, D, H).transpose(0, 2, 1).reshape(C, C)
    )


def _headmajor_b(bv):
    return np.ascontiguousarray(bv.reshape(D, H).T.reshape(C))


def prepare_in_maps(hidden_state, mask, Wq, bq, Wk, bk, Wv, bv, Wo, bo):
    x = np.asarray(hidden_state).reshape(B, C, S)
    m = np.asarray(mask).reshape(B, S, S)
    scale = np.float32(D**-0.5)

    wqt = np.ascontiguousarray(
        (_headmajor(np.asarray(Wq).T).astype(np.float32) * scale).astype(np.float16)
    )
    wkt = _headmajor(np.asarray(Wk).T)
    wvt = _headmajor(np.asarray(Wv).T)
    wot = np.ascontiguousarray(np.asarray(Wo).T)

    bq_s = (_headmajor_b(np.asarray(bq)).astype(np.float32) * scale).astype(
        np.float32
    )
    bk_p = np.asarray(bk).astype(np.float32)
    bk_p = _headmajor_b(bk_p)
    # fold bv through attention (softmax weights sum to 1) into bo:
    # bo' = bo + Wo @ bv_headmajor
    bv_hm = _headmajor_b(np.asarray(bv).astype(np.float32))
    bo_p = np.asarray(bo).astype(np.float32) + np.asarray(Wo).astype(
        np.float32
    ) @ bv_hm
    bcols = np.zeros((128, 19), dtype=np.float32)
    for j in range(NC_CHUNKS):
        bcols[:, j] = bq_s[j * 128 : (j + 1) * 128]
        bcols[:, 6 + j] = bk_p[j * 128 : (j + 1) * 128]
        bcols[:, 12 + j] = bo_p[j * 128 : (j + 1) * 128]
    bcols[:, 18] = EM_BIAS

    shared = {
        "wqt": wqt,
        "wkt": wkt,
        "wvt": wvt,
        "wot": wot,
        "bcols": np.ascontiguousarray(bcols),
    }
    in_maps = []
    for i in range(NCORES):
        sl = slice(i * NB, (i + 1) * NB)
        in_maps.append(
            dict(
                shared,
                x=np.ascontiguousarray(x[sl]),
                mask=np.ascontiguousarray(m[sl]),
            )
        )
    return in_maps


def kernel(**inputs):
    nc = _get_compiled()
    in_maps = prepare_in_maps(**inputs)
    res = run_bass_kernel_spmd(nc, in_maps, core_ids=list(range(NCORES)))
    y = np.concatenate([res.results[i]["y"] for i in range(NCORES)], axis=0)
    return y.reshape(B, C, 1, S)
